# revision 20
# baseline (speedup 1.0000x reference)
"""Trainium2 Bass kernel for BitFlipLinear: y[b,s,o] = sum_i x[b,s,i]*W[o,i] + bias[o].

Data-parallel over batch: each of the 8 NeuronCores computes one
[4096,4096] @ [4096,4096]^T matmul (137 GFLOP/core).

Precision: W's values {0,1,3} are exact in BOTH bf16 and fp8(e4m3).
 - x = x_hi + x_lo with x_hi = bf16(x), x_lo = bf16(x - x_hi): two bf16
   matmul passes accumulated in fp32 PSUM give effectively fp32-accurate
   output (rel err ~2.5e-6) at bf16 TensorE speed.
 - W^T is kept FULLY resident in SBUF as fp8 (128 KB/partition); the PE
   accepts a bf16 stationary x fp8 moving matmul, and since W is exact in
   fp8 this loses no precision (verified on HW).  Full residency removes
   any o-tiling of the weight and streams x^T only once.

Per-core pipeline (single NEFF, Tile-scheduled):
  - cast W fp32->bf16 in DRAM (SWDGE cast-DMA, ~345 GB/s)
  - fill wres8 = W^T fp8 [128, 32, 4096] via XBAR transpose-DMA chunks
    (bf16) + DVE cast to fp8
  - per 128-row s-tile: DVE-split x into x_hi/x_lo bf16 (DRAM scratch),
    read back transposed via XBAR transpose-DMA; run 8 PSUM-bank chains
    (ob-outer, ko-inner) in two alternating 4-bank PSUM slots so eviction
    overlaps compute; the bias (partition-replicated once via K=2
    matmuls) is added during the PSUM->SBUF eviction on the
    otherwise-idle Vector/Scalar engines.
"""

import os
import sys

for _p in ("/opt/trn_rl_repo",):
    if os.path.isdir(_p) and _p not in sys.path:
        sys.path.append(_p)

import numpy as np

B, S, K, O = 8, 4096, 4096, 4096
N_CORES = 8
CONV_I = 256
W_CAST_ROWS = 512
BIAS_CH = 256
W8_CH = 64              # o-cols per W^T fp8 fill chunk (transpose+cast)

_NC_CACHE = {}


def build_nc(S=S, K=K, O=O, enable_asserts=False, repeat=1):
    import concourse.bacc as bacc
    import concourse.tile as tile
    import concourse.mybir as mybir

    f32 = mybir.dt.float32
    bf16 = mybir.dt.bfloat16
    fp8 = mybir.dt.float8e4
    P = 128
    NB = O // 512             # total PSUM-bank chains per s-tile (8)
    ST = S // P
    KO = K // P
    NCV = K // CONV_I
    HALF_BANKS = NB // 2      # 4 banks per psum slot

    nc = bacc.Bacc("TRN2", target_bir_lowering=False, debug=False,
                   enable_asserts=enable_asserts)

    ap_x = nc.dram_tensor("x", [S, K], f32, kind="ExternalInput").ap()
    ap_w = nc.dram_tensor("w", [O, K], f32, kind="ExternalInput").ap()
    ap_bias = nc.dram_tensor("bias", [O], f32, kind="ExternalInput").ap()
    ap_y = nc.dram_tensor("y", [S, O], f32, kind="ExternalOutput").ap()

    with tile.TileContext(nc) as tc:
        with (
            tc.tile_pool(name="dram", bufs=1, space="DRAM") as dram,
            tc.tile_pool(name="const", bufs=1) as const,
            tc.tile_pool(name="bstage", bufs=1) as bstage,
            tc.tile_pool(name="convin", bufs=2) as convin,
            tc.tile_pool(name="convout", bufs=2) as convout,
            tc.tile_pool(name="wres", bufs=1) as wresp,
            tc.tile_pool(name="wstg", bufs=2) as wstg,
            tc.tile_pool(name="xts", bufs=2) as xtsp,
            tc.tile_pool(name="outp", bufs=2) as outp,
            tc.tile_pool(name="psum", bufs=2, space="PSUM") as psum,
        ):
            wb = dram.tile([O, K], bf16)
            xhi = dram.tile([S, K], bf16)
            xlo = dram.tile([S, K], bf16)

            for _rep in range(repeat):
              # bias2[0]=bf16(bias), bias2[1]=bf16(bias-hi)
              bias2 = const.tile([2, O], bf16)
              ones2 = const.tile([2, P], bf16)
              nc.vector.memset(ones2[:], 1.0)
              bch = min(BIAS_CH, O)
              for c in range(O // bch):
                sl = slice(c * bch, (c + 1) * bch)
                bst = bstage.tile([1, bch], f32)
                nc.scalar.dma_start(bst[:], ap_bias[None, sl])
                nc.vector.tensor_copy(bias2[0:1, sl], bst[:])
                blo = bstage.tile([1, bch], bf16, tag="blo")
                nc.vector.tensor_sub(blo[:], bst[:], bias2[0:1, sl])
                nc.scalar.dma_start(bias2[1:2, sl], blo[:])

              # W fp32 -> bf16 in DRAM
              wch = min(W_CAST_ROWS, O)
              for c in range(O // wch):
                sl = slice(c * wch, (c + 1) * wch)
                nc.gpsimd.dma_start(wb[sl, :], ap_w[sl, :])

              # partition-replicated bias (fp16): bias_rep[p, o] = bias[o]
              # built once via K=2 matmuls (ones^T @ [bias_hi; bias_lo])
              bias_rep = const.tile([P, O], mybir.dt.float16)
              grp = min(HALF_BANKS, O // 512)
              for g in range(O // (grp * 512)):
                  bp = psum.tile([P, grp * 512], f32, tag="pt")
                  for obl in range(grp):
                      b0 = (g * grp + obl) * 512
                      nc.tensor.matmul(
                          bp[:, obl * 512:(obl + 1) * 512],
                          ones2[:], bias2[:, b0:b0 + 512],
                          start=True, stop=True,
                      )
                  nc.vector.tensor_copy(
                      bias_rep[:, g * grp * 512:(g + 1) * grp * 512], bp[:])

              # full W^T resident as fp8: wres8[pi, ko, o] = W[o, ko*128+pi]
              wres8 = wresp.tile([P, KO, O], fp8)
              for c in range(O // W8_CH):
                osl = slice(c * W8_CH, (c + 1) * W8_CH)
                stg = wstg.tile([P, KO, W8_CH], bf16)
                nc.sync.dma_start(stg[:], wb[osl, :], transpose=True)
                nc.vector.tensor_copy(wres8[:, :, osl], stg[:])

              for st in range(ST):
                rows = slice(st * P, (st + 1) * P)
                # split x rows into hi/lo bf16 (DRAM scratch, read back transposed)
                for c in range(NCV):
                    cols = slice(c * CONV_I, (c + 1) * CONV_I)
                    xin = convin.tile([P, CONV_I], f32)
                    nc.scalar.dma_start(xin[:], ap_x[rows, cols])
                    ch = convout.tile([P, 2, CONV_I], bf16)
                    nc.vector.tensor_copy(ch[:, 0], xin[:])
                    nc.vector.tensor_sub(ch[:, 1], xin[:], ch[:, 0])
                    nc.scalar.dma_start(xhi[rows, cols], ch[:, 0])
                    nc.scalar.dma_start(xlo[rows, cols], ch[:, 1])

                xt = xtsp.tile([P, 2, KO, P], bf16)
                nc.sync.dma_start(xt[:, 0], xhi[rows, :], transpose=True)
                nc.sync.dma_start(xt[:, 1], xlo[rows, :], transpose=True)

                # two psum slots of 4 banks each; chains staggered ob-outer
                for half in range(2):
                    pt = psum.tile([P, HALF_BANKS * 512], f32)
                    for obl in range(HALF_BANKS):
                        ob = half * HALF_BANKS + obl
                        b0 = ob * 512
                        bank = pt[:, obl * 512:(obl + 1) * 512]
                        for ko in range(KO):
                            for h in range(2):
                                first = (ko == 0) and (h == 0)
                                last = (ko == KO - 1) and (h == 1)
                                nc.tensor.matmul(
                                    bank,
                                    xt[:, h, ko, :],
                                    wres8[:, ko, b0:b0 + 512],
                                    start=first, stop=last,
                                )
                    ot = outp.tile([P, HALF_BANKS * 512], f32)
                    # bias folded into the eviction (engines otherwise idle)
                    o0h = half * HALF_BANKS * 512
                    nc.any.tensor_add(
                        ot[:], pt[:], bias_rep[:, o0h:o0h + HALF_BANKS * 512])
                    o0 = half * HALF_BANKS * 512
                    nc.scalar.dma_start(
                        ap_y[rows, o0:o0 + HALF_BANKS * 512], ot[:]
                    )

    nc.compile()
    return nc


def _get_nc():
    key = (S, K, O)
    if key not in _NC_CACHE:
        _NC_CACHE[key] = build_nc(S, K, O)
    return _NC_CACHE[key]


def make_in_maps(x, weight, bias):
    x = np.ascontiguousarray(np.asarray(x, dtype=np.float32))
    weight = np.ascontiguousarray(np.asarray(weight, dtype=np.float32))
    bias = np.ascontiguousarray(np.asarray(bias, dtype=np.float32))
    assert x.shape == (B, S, K), x.shape
    return [
        {"x": np.ascontiguousarray(x[b]), "w": weight, "bias": bias}
        for b in range(B)
    ]


def kernel(x, weight, bias):
    from concourse.bass_utils import run_bass_kernel_spmd

    nc = _get_nc()
    in_maps = make_in_maps(x, weight, bias)
    res = run_bass_kernel_spmd(nc, in_maps, core_ids=list(range(N_CORES)))
    return np.stack([res.results[b]["y"] for b in range(B)], axis=0).astype(np.float32)
